# revision 12
# baseline (speedup 1.0000x reference)
"""Trainium2 Bass kernel for the FCBlock weight-transform + matmul problem.

Math (per reference):
    W_i = per-head 3x3 conv over W.reshape(4, 1024, 4096) + conv_b
          + sigmoid(sk_wt) * W            (per-head scalars)
    out  = inp @ W_i.T                    (inp: [2, 2048, 4096])

Strategy: tensor-parallel shard of W_i along fout across 8 NeuronCores
(512 fout columns each, inside one head).  Host-side prep (layout only):
inp is transposed/cast to fp8-e4m3 xT [fin, tok] so the contraction dim
lands on partitions with zero on-device transposes; W is shipped as a
transposed bf16 slice with conv halos so the weight transform runs in
the transposed domain and emits W_i^T directly.

On each core:
  - transform: banded [128,128] matrices (built from conv_w/sk_wt) run
    the 3x3 conv as PE band-matmuls over W^T windows, + a 6-row halo
    matmul; PSUM result is scaled x16 and cast to fp8 (bias withheld).
  - main matmul: fp8 DoubleRow (2 k-groups per instr, 2x PE rate)
    over [fin,tok] x [fin,fout] tiles, fp32 PSUM; the conv bias is
    restored as a rank-1 update b*rowsum(inp) during the PSUM drain.
Output is sharded on fout; the host concatenates.
"""

import numpy as np
import ml_dtypes

import concourse.bass as bass
import concourse.mybir as mybir
import concourse.tile as tile
from concourse import bacc
from concourse.bass_utils import run_bass_kernel_spmd

F32 = mybir.dt.float32
BF16 = mybir.dt.bfloat16
FP8 = mybir.dt.float8e4
DR = mybir.MatmulPerfMode.DoubleRow

NCORES = 8
NUM_HEADS = 4
TOK = 4096          # 2 * 2048 tokens
FIN = 4096
FOUT = 4096
FSH = FOUT // NCORES  # 512 fout columns per core
WSCALE = 16.0         # fp8 pre-scale on W_i (drained as x16, undone on out)


def build_program(tok=TOK, fin=FIN):
    assert tok % 512 == 0 and fin % 256 == 0
    n_sb = tok // 512            # 512-token superblocks
    n_win = fin // 128           # fin windows (transform) == k-blocks
    n_kp = fin // 256            # DoubleRow k-pairs

    nc = bacc.Bacc(None, target_bir_lowering=False)

    xt8 = nc.declare_dram_parameter("xt8", [fin, tok], FP8, isOutput=False)
    wth = nc.declare_dram_parameter("wth", [fin + 2, FSH + 2], BF16,
                                    isOutput=False)
    # s2x packs per-token rowsums [:, :tok//128] and the 11 transform
    # scalars ([0, tok//128:]) into one param -> one DMA at startup
    s2x = nc.declare_dram_parameter("s2x", [128, tok // 128 + 11], F32,
                                    isOutput=False)
    out = nc.declare_dram_parameter("o", [tok, FSH], F32, isOutput=True)

    with tile.TileContext(nc) as tc:
        with (
            tc.tile_pool(name="const", bufs=1) as const,
            tc.tile_pool(name="wt8p", bufs=1) as wt8p,
            tc.tile_pool(name="wfp", bufs=4) as wfp,
            tc.tile_pool(name="hfp", bufs=1) as hfp,
            tc.tile_pool(name="xb", bufs=3) as xbp,
            tc.tile_pool(name="osb", bufs=4) as osbp,
            tc.tile_pool(name="psw", bufs=4, space="PSUM") as psw,
            tc.tile_pool(name="psx", bufs=4, space="PSUM") as psx,
        ):
            nT = tok // 128
            wt_raw = wth.tensor if hasattr(wth, "tensor") else wth

            # ---- kick off all transform DMAs first (sync issues in
            # program order; these land while the scalar setup runs) -----
            wfall = []
            for g in range(n_win // 8):
                wf = wfp.tile([128, 8, FSH + 2], BF16, tag="wf")
                nc.sync.dma_start(
                    out=wf[:],
                    in_=bass.AP(wt_raw, (128 * 8 * g + 1) * (FSH + 2),
                                [[FSH + 2, 128], [128 * (FSH + 2), 8],
                                 [1, FSH + 2]]))
                wfall.append(wf)
            # halo rows for all windows: partition = (edge, dr), free (w, n)
            hhall = hfp.tile([6, n_win, FSH], BF16, tag="hh")
            for e in range(2):
                nc.sync.dma_start(
                    out=hhall[3 * e:3 * e + 3, :, :],
                    in_=bass.AP(wt_raw, e * 129 * (FSH + 2),
                                [[1, 3], [128 * (FSH + 2), n_win],
                                 [1, FSH]]))

            # ---- setup: scalars, band + halo matrices -------------------
            s2_sb = const.tile([128, nT + 11], F32)
            nc.sync.dma_start(out=s2_sb[:], in_=s2x[:])
            sc_sb = s2_sb[0:1, nT:nT + 11]

            ones_r = const.tile([1, 128], F32)
            nc.vector.memset(ones_r[:], 1.0)

            # warm-up: ~8 zero matmuls keep the PE busy through the HAM
            # window so the transform starts at 2.4 GHz
            zt = const.tile([128, FSH], BF16)
            nc.vector.memset(zt[:], 0.0)
            for i in range(8):
                pz = psx.tile([128, FSH], F32, tag="px")
                nc.tensor.matmul(pz[:], zt[:, 0:128], zt[:],
                                 start=True, stop=True)

            # broadcast the 11 scalars to all 128 partitions via k=1 matmul
            ps_b = psw.tile([128, 11], F32, tag="pw")
            nc.tensor.matmul(ps_b[:], ones_r[:], s2_sb[0:1, nT:nT + 11],
                             start=True, stop=True)
            scv = const.tile([128, 11], F32)
            nc.vector.tensor_copy(out=scv[:], in_=ps_b[:])

            # bS[p, T] = conv_b * rowsum(inp)[128T + p]
            bS = const.tile([128, nT], F32)
            nc.vector.tensor_scalar(bS[:], s2_sb[:, 0:nT], scv[:, 9:10],
                                    None, mybir.AluOpType.mult)

            # ctr = conv_w[h,1,1] + sigmoid(sk_wt[h])
            sig = const.tile([128, 1], F32)
            nc.scalar.activation(sig[:], scv[:, 10:11],
                                 mybir.ActivationFunctionType.Sigmoid)
            ctr = const.tile([128, 1], F32)
            nc.vector.tensor_tensor(out=ctr[:], in0=sig[:], in1=scv[:, 4:5],
                                    op=mybir.AluOpType.add)

            # diagonal masks for bands k-c in {-1, 0, +1}
            masks = {}
            for d in (-1, 0, 1):
                m = const.tile([128, 128], F32, tag=f"mask{d}")
                nc.gpsimd.memset(m[:], 0.0)
                nc.gpsimd.affine_select(
                    out=m[:], in_=m[:],
                    compare_op=mybir.AluOpType.not_equal,
                    fill=1.0, base=-d, channel_multiplier=1,
                    pattern=[[-1, 128]],
                )
                masks[d] = m

            # M_dr[k, c] = cw[dr, k-c+1]; center band of dr=1 adds sigmoid
            m_dr = []
            for dr in range(3):
                mf = const.tile([128, 128], F32, tag=f"mf{dr}")
                nc.vector.tensor_scalar(mf[:], masks[-1][:],
                                        scv[:, 3 * dr:3 * dr + 1], None,
                                        mybir.AluOpType.mult)
                mid = ctr if dr == 1 else scv[:, 3 * dr + 1:3 * dr + 2]
                nc.vector.scalar_tensor_tensor(
                    out=mf[:], in0=masks[0][:], scalar=mid, in1=mf[:],
                    op0=mybir.AluOpType.mult, op1=mybir.AluOpType.add)
                nc.vector.scalar_tensor_tensor(
                    out=mf[:], in0=masks[1][:],
                    scalar=scv[:, 3 * dr + 2:3 * dr + 3], in1=mf[:],
                    op0=mybir.AluOpType.mult, op1=mybir.AluOpType.add)
                mb = const.tile([128, 128], BF16, tag=f"mb{dr}")
                nc.vector.tensor_copy(out=mb[:], in_=mf[:])
                m_dr.append(mb)

            # halo matrix Mh [6, 128]: partitions (top/bot halo x 3 dr);
            # top halo row feeds out c=0 with cw[dr,0], bottom feeds c=127
            # with cw[dr,2].  Built as outer products v.T @ onehot.
            onehot0 = const.tile([1, 128], F32)
            nc.vector.memset(onehot0[:], 0.0)
            nc.vector.memset(onehot0[:, 0:1], 1.0)
            onehot127 = const.tile([1, 128], F32)
            nc.vector.memset(onehot127[:], 0.0)
            nc.vector.memset(onehot127[:, 127:128], 1.0)
            v_a = const.tile([1, 6], F32)
            nc.vector.memset(v_a[:], 0.0)
            v_b = const.tile([1, 6], F32)
            nc.vector.memset(v_b[:], 0.0)
            for dr in range(3):
                nc.vector.tensor_copy(
                    out=v_a[:, dr:dr + 1],
                    in_=s2_sb[0:1, nT + 3 * dr:nT + 3 * dr + 1])
                nc.vector.tensor_copy(
                    out=v_b[:, 3 + dr:4 + dr],
                    in_=s2_sb[0:1, nT + 3 * dr + 2:nT + 3 * dr + 3])
            ps6 = psw.tile([6, 128], F32, tag="pw")
            nc.tensor.matmul(ps6[:], v_a[:], onehot0[:], start=True,
                             stop=False)
            nc.tensor.matmul(ps6[:], v_b[:], onehot127[:], start=False,
                             stop=True)
            h6 = const.tile([6, 128], BF16)
            nc.vector.tensor_copy(out=h6[:], in_=ps6[:])

            # ---- phase T: weight transform -> W_i^T (fp8, x16) ----------
            wt8 = wt8p.tile([128, n_win, FSH], FP8)
            for w in range(n_win):
                wf = wfall[w // 8]
                wi = w % 8
                pw = psw.tile([128, FSH], F32, tag="pw")
                for dr in range(3):
                    nc.tensor.matmul(pw[:], m_dr[dr][:],
                                     wf[:, wi, dr:dr + FSH],
                                     start=(dr == 0), stop=False)
                nc.tensor.matmul(pw[:], h6[:], hhall[:, w, :],
                                 start=False, stop=True)
                if w % 2 == 0:
                    nc.scalar.mul(wt8[:, w, :], pw[:], WSCALE)
                else:
                    nc.vector.tensor_scalar(wt8[:, w, :], pw[:], WSCALE,
                                            None, mybir.AluOpType.mult)

            # ---- phase M: fp8 DoubleRow main matmul ---------------------
            xt_raw = xt8.tensor if hasattr(xt8, "tensor") else xt8
            for sb in range(n_sb):
                xb = xbp.tile([128, n_win, 512], FP8, tag="xb")
                nc.sync.dma_start(
                    out=xb[:],
                    in_=bass.AP(xt_raw, 512 * sb,
                                [[tok, 128], [128 * tok, n_win], [1, 512]]))
                for tb in range(4):
                    T = 4 * sb + tb
                    po = psx.tile([128, FSH], F32, tag="px")
                    for kp in range(n_kp):
                        nc.tensor.matmul(
                            po[:],
                            xb[:, 2 * kp:2 * kp + 2, 128 * tb:128 * tb + 128],
                            wt8[:, 2 * kp:2 * kp + 2, :],
                            start=(kp == 0), stop=(kp == n_kp - 1),
                            perf_mode=DR)
                    ob = osbp.tile([128, FSH], F32, tag="ob")
                    if T % 2 == 0:
                        nc.scalar.activation(
                            ob[:], po[:], mybir.ActivationFunctionType.Identity,
                            bias=bS[:, T:T + 1], scale=1.0 / WSCALE)
                    else:
                        nc.vector.tensor_scalar(
                            ob[:], po[:], 1.0 / WSCALE, bS[:, T:T + 1],
                            mybir.AluOpType.mult, mybir.AluOpType.add)
                    nc.sync.dma_start(out=out[128 * T:128 * T + 128, :],
                                      in_=ob[:])

    nc.compile()
    return nc


def shard_inputs(inp, W, conv_w, conv_b, sk_wt, fin=FIN):
    """Build the 8 per-core input maps (host-side layout prep only)."""
    tok = inp.size // fin
    e4 = ml_dtypes.float8_e4m3
    x2 = np.asarray(inp, dtype=np.float32).reshape(tok, fin)
    xt8 = np.ascontiguousarray(x2.T).astype(e4)          # [fin, tok] fp8
    WT = np.asarray(W, dtype=np.float32).T               # [fin, fout]
    hsz = W.shape[0] // NUM_HEADS
    conv_w = np.asarray(conv_w, dtype=np.float32)
    conv_b = np.asarray(conv_b, dtype=np.float32)
    sk_wt = np.asarray(sk_wt, dtype=np.float32)

    in_maps = []
    for c in range(NCORES):
        o0 = c * FSH
        h = o0 // hsz
        wth = np.zeros((fin + 2, FSH + 2), dtype=ml_dtypes.bfloat16)
        wth[1:fin + 1, 1:FSH + 1] = WT[:, o0:o0 + FSH].astype(
            ml_dtypes.bfloat16)
        if o0 % hsz != 0:          # left fout-halo stays inside the head
            wth[1:fin + 1, 0] = WT[:, o0 - 1].astype(ml_dtypes.bfloat16)
        if (o0 + FSH) % hsz != 0:  # right fout-halo stays inside the head
            wth[1:fin + 1, FSH + 1] = WT[:, o0 + FSH].astype(
                ml_dtypes.bfloat16)
        s2x = np.zeros((128, tok // 128 + 11), dtype=np.float32)
        s2x[:, :tok // 128] = (
            x2.sum(axis=1, dtype=np.float64).astype(np.float32)
            .reshape(tok // 128, 128).T)
        s2x[0, tok // 128:tok // 128 + 9] = conv_w[h].reshape(9)
        s2x[0, tok // 128 + 9] = conv_b[h]
        s2x[0, tok // 128 + 10] = sk_wt[h].reshape(())
        in_maps.append({"xt8": xt8, "wth": wth, "s2x": s2x})
    return in_maps


_PROGRAM_CACHE = {}


def _get_program(tok=TOK, fin=FIN):
    key = (tok, fin)
    if key not in _PROGRAM_CACHE:
        _PROGRAM_CACHE[key] = build_program(tok, fin)
    return _PROGRAM_CACHE[key]


def kernel(inp, W, conv_w, conv_b, sk_wt):
    nc = _get_program(TOK, FIN)
    in_maps = shard_inputs(inp, W, conv_w, conv_b, sk_wt)
    res = run_bass_kernel_spmd(nc, in_maps, list(range(NCORES)))
    shards = [res.results[c]["o"].reshape(2, TOK // 2, FSH)
              for c in range(NCORES)]
    return np.ascontiguousarray(
        np.concatenate(shards, axis=-1).astype(np.float32))
